# revision 1
# baseline (speedup 1.0000x reference)
"""Causal multi-head attention with RoPE on 8 Trainium2 NeuronCores.

Problem: B=2, N=2048, DIM=1024, H=16, DH=64, fp32 in/out.
Sharding: head-parallel - core c owns heads 2c, 2c+1 (columns c*128:(c+1)*128
of Wq/Wk/Wv, rows c*128:(c+1)*128 of Wo) for both batches. Each core computes
its partial output projection [DIM, B*N] in fp16; the host sums the 8
partials (the "all-reduce") and adds the bias.

v2 vs baseline:
  - fp16 end-to-end on SBUF/HBM (fp32 only in PSUM accumulation): halves DMA
    and enables 2x DVE modes on SBUF-side elementwise ops.
  - V transposed per 128-token block as one [128,128] PE transpose covering
    both heads; v_aug interleaves [h0 64 | ones | h1 64 | ones] so one
    strided DVE copy scatters both heads.
  - causal diagonal handled at 128-col granularity: S/exp/PV skip the
    fully-masked region, mask add is a N=128 matmul with a shared [128,128]
    triangle pattern.
  - S matmuls for the two heads (K=64, partition bases 0/64) are emitted
    adjacently so they pack into disjoint PE row groups and run concurrently.
  - batch-0/batch-1 attention chunks interleave with QKV chunks and deferred
    Wo projections so PE always has queued work while ACT runs exp; each
    attention block is preceded in the PE queue by an independent QKV chunk
    so S matmuls never head-of-line block on the RoPE chain.
  - q/k PSUM evacuation on ScalarE; rotate-half via partition-shifted DVE
    copies; x chunks prefetched 3 slots ahead; one output store DMA per
    4 Wo blocks.

HW-validated rel err vs fp32 reference: 4.7e-04 (budget 2e-2).
"""
import numpy as np
import bass_rust
from concourse import bacc
import concourse.mybir as mybir
from concourse.tile import TileContext
from concourse.bass_utils import run_bass_kernel_spmd

B, N, DIM, H, DH = 2, 2048, 1024, 16, 64
NCORES = 8
HPC = H // NCORES          # 2 heads per core
T = B * N                  # 4096 tokens
CHUNK = 512
NCH = T // CHUNK           # 8 token chunks
NCB = DIM // 128           # 8 contraction blocks
NJB = N // 128             # 16 j-blocks per batch
NEG = -60000.0             # causal mask add (fp16-safe; exp -> 0 in fp32)

F32 = mybir.dt.float32
F16 = mybir.dt.float16

_NC_CACHE = {}


def build(reps=1):
    nc = bacc.Bacc()
    xTD = nc.dram_tensor("xT", [DIM, T], F16, kind="ExternalInput")
    wqD = nc.dram_tensor("wq", [DIM, 128], F16, kind="ExternalInput")
    wkD = nc.dram_tensor("wk", [DIM, 128], F16, kind="ExternalInput")
    wvD = nc.dram_tensor("wv", [DIM, 128], F16, kind="ExternalInput")
    woD = nc.dram_tensor("wo", [128, DIM], F16, kind="ExternalInput")
    cosD = nc.dram_tensor("cosT", [128, N], F16, kind="ExternalInput")
    sinsD = nc.dram_tensor("sinsT", [128, N], F16, kind="ExternalInput")
    identD = nc.dram_tensor("identD", [128, 128], F16, kind="ExternalInput")
    identFD = nc.dram_tensor("identFD", [128, 128], F32, kind="ExternalInput")
    maskD = nc.dram_tensor("maskD", [128, 128], F16, kind="ExternalInput")
    outD = nc.dram_tensor("outT", [DIM, T], F16, kind="ExternalOutput")

    Exp = mybir.ActivationFunctionType.Exp

    with TileContext(nc) as tc:
        with (
            tc.tile_pool(name="const", bufs=1) as cp,
            tc.tile_pool(name="sb", bufs=2) as sb,
            tc.tile_pool(name="ps", bufs=1, space="PSUM") as ps,
        ):
            ident = cp.tile([128, 128], F16)
            identF = cp.tile([128, 128], F32)
            mask128 = cp.tile([128, 128], F16)
            wq = cp.tile([128, NCB, 128], F16)
            wk = cp.tile([128, NCB, 128], F16)
            wv = cp.tile([128, NCB, 128], F16)
            wo = cp.tile([128, NCB, 128], F16)
            cos2 = cp.tile([128, N], F16)
            sins2 = cp.tile([128, N], F16)
            qkt = cp.tile([128, 2, T], F16)   # slot 0 = q, slot 1 = k
            v_aug = cp.tile([128, B, NJB, 130], F16)

            xt_tiles = {}

            def issue_xt(ch):
                """Prefetch one 512-token x chunk as two half DMAs."""
                t0 = ch * CHUNK
                xt = sb.tile([128, NCB, CHUNK], F16, tag="xt", bufs=4,
                             name=f"xt_c{ch}")
                half = NCB // 2
                for hb in range(2):
                    nc.sync.dma_start(
                        out=xt[:, hb * half:(hb + 1) * half, :],
                        in_=xTD[hb * half * 128:(hb + 1) * half * 128,
                                t0:t0 + CHUNK].rearrange(
                            "(cb p) n -> p cb n", p=128))
                xt_tiles[ch] = xt

            # prologue: wq first (first matmul), then the tiny mask
            # constants (A00's mask matmuls and PV need them early), then
            # x/weights in consumption order
            for hb in range(2):
                nc.sync.dma_start(
                    out=wq[:, hb * 4:(hb + 1) * 4, :],
                    in_=wqD[hb * 512:(hb + 1) * 512, :].rearrange(
                        "(cb p) d -> p cb d", p=128))
            nc.sync.dma_start(out=ident, in_=identD[:])
            nc.sync.dma_start(out=mask128, in_=maskD[:])
            nc.vector.memset(
                v_aug.rearrange("p b j (two c) -> p b j two c", two=2)
                [:, :, :, :, 64:65], 1.0)
            issue_xt(0)
            nc.sync.dma_start(out=cos2[:, 0:CHUNK], in_=cosD[:, 0:CHUNK])
            nc.sync.dma_start(out=sins2[:, 0:CHUNK], in_=sinsD[:, 0:CHUNK])
            nc.sync.dma_start(
                out=wk, in_=wkD[:].rearrange("(cb p) d -> p cb d", p=128))
            issue_xt(4)
            nc.sync.dma_start(
                out=wv, in_=wvD[:].rearrange("(cb p) d -> p cb d", p=128))
            issue_xt(1)
            nc.sync.dma_start(out=cos2[:, CHUNK:], in_=cosD[:, CHUNK:])
            nc.sync.dma_start(out=sins2[:, CHUNK:], in_=sinsD[:, CHUNK:])
            nc.sync.dma_start(out=identF, in_=identFD[:])
            nc.gpsimd.dma_start(
                out=wo, in_=woD[:].rearrange("p (db d) -> p db d", d=128))

            # x chunks are consumed in order 0,4,1,5,2,6,3,7; prefetch the
            # chunk-after-next at the start of each p1 chunk
            _xt_order = [0, 4, 1, 5, 2, 6, 3, 7]

            def p1_chunk(ch):
                """QKV + RoPE + V transpose for one 512-token chunk."""
                t0 = ch * CHUNK
                bidx = t0 // N
                csl = slice(t0 % N, t0 % N + CHUNK)   # cos/sin columns
                nxt = _xt_order.index(ch) + 3
                if nxt < NCH:
                    issue_xt(_xt_order[nxt])
                xt = xt_tiles.pop(ch)
                # q and k projections share one 2-bank PSUM tile
                pp = ps.tile([128, 2, CHUNK], F32, tag="big", bufs=2,
                             name=f"ppqk{ch}")
                for sl, W in ((0, wq), (1, wk)):
                    for cb in range(NCB):
                        nc.tensor.matmul(pp[:, sl, :], W[:, cb, :],
                                         xt[:, cb, :],
                                         start=(cb == 0), stop=(cb == NCB - 1))
                raw = sb.tile([128, 2, CHUNK], F16, tag="raw", bufs=4,
                              name=f"raw{ch}")
                nc.scalar.copy(raw, pp)
                # rotate-half swap (sign folded into sins): partition-shifted
                # DVE copies covering both q and k slots
                rawsw = sb.tile([128, 2, CHUNK], F16, tag="rawsw", bufs=4,
                                name=f"rawsw{ch}")
                for hh in (0, 64):
                    a, bnd, c2 = hh, hh + 32, hh + 64
                    nc.vector.tensor_copy(rawsw[a:bnd], raw[bnd:c2])
                    nc.vector.tensor_copy(rawsw[bnd:c2], raw[a:bnd])
                # v projection (overlaps the q/k RoPE elementwise chain)
                ppv = ps.tile([128, 2, CHUNK], F32, tag="big", bufs=2,
                              name=f"ppv{ch}")
                for cb in range(NCB):
                    nc.tensor.matmul(ppv[:, 0, :], wv[:, cb, :], xt[:, cb, :],
                                     start=(cb == 0), stop=(cb == NCB - 1))
                # evacuate v early so the ppv PSUM buffer frees before
                # the RoPE chain drains through the DVE queue
                vtc = sb.tile([128, CHUNK], F32, tag="vtc", bufs=2,
                              name=f"vtc{ch}")
                nc.vector.tensor_copy(vtc, ppv[:, 0, :])
                # RoPE: qkt = raw*cos + rawsw*sins
                for sl in range(2):
                    tmp = sb.tile([128, CHUNK], F16, tag="tmp", bufs=2,
                                  name=f"tmp{ch}{sl}")
                    tmp2 = sb.tile([128, CHUNK], F16, tag="tmp2", bufs=2,
                                   name=f"tmp2{ch}{sl}")
                    nc.vector.tensor_mul(tmp, raw[:, sl, :], cos2[:, csl])
                    nc.vector.tensor_mul(tmp2, rawsw[:, sl, :], sins2[:, csl])
                    nc.vector.tensor_add(qkt[:, sl, t0:t0 + CHUNK], tmp, tmp2)
                for tb in range(4):
                    jb = (ch % 4) * 4 + tb
                    pt = ps.tile([128, CHUNK], F32, tag="sm", bufs=2,
                                 name=f"pt{ch}{tb}")
                    nc.tensor.transpose(
                        pt[:, 0:128], vtc[:, tb * 128:(tb + 1) * 128],
                        identF)
                    nc.vector.tensor_copy(
                        v_aug[:, bidx, jb, :].rearrange(
                            "p (two c) -> p two c", two=2)[:, :, 0:64],
                        pt[:, 0:128].rearrange("p (two c) -> p two c", two=2))

            def p2_attn(bidx, ch):
                """S/exp/PV + softmax normalize for i-chunk ch of batch bidx.

                Returns the normalized attention output tile [128, CHUNK]
                (rows 0:64 head0, 64:128 head1) for the deferred projection.
                """
                gcol = bidx * N + ch * CHUNK
                njb = 4 * (ch + 1)
                pos = []
                for h in range(HPC):
                    pos.append(ps.tile([DH + 1, CHUNK], F32, tag="po", bufs=2,
                                       name=f"po{bidx}{ch}{h}"))
                expts = []
                for jb in range(njb):
                    r = jb - 4 * ch          # >=0 on the diagonal band
                    i0 = 128 * r if r > 0 else 0
                    jc = bidx * N + jb * 128
                    pst = ps.tile([128, 2, CHUNK], F32, tag="big", bufs=2,
                                  name=f"ps{bidx}{ch}{jb}")
                    diag = r >= 0
                    # head0 (rows 0:64) and head1 (rows 64:128) S matmuls
                    # pack into disjoint PE row groups
                    for h in range(HPC):
                        qr = slice(h * 64, (h + 1) * 64)
                        nc.tensor.matmul(
                            pst[:, h, i0:CHUNK], qkt[qr, 1, jc:jc + 128],
                            qkt[qr, 0, gcol + i0:gcol + CHUNK],
                            start=True, stop=not diag)
                    if diag:
                        for h in range(HPC):
                            nc.tensor.matmul(
                                pst[:, h, i0:i0 + 128], ident, mask128,
                                start=False, stop=True)
                    expt = sb.tile([128, 2, CHUNK], F16, tag="expt", bufs=8,
                                   name=f"e{bidx}{ch}{jb}")
                    nc.scalar.activation(expt[:, :, i0:CHUNK],
                                         pst[:, :, i0:CHUNK], Exp)
                    expts.append((jb, i0, expt))
                    for h in range(HPC):
                        nc.tensor.matmul(
                            pos[h][:, i0:CHUNK],
                            v_aug[:, bidx, jb, 65 * h:65 * h + 65],
                            expt[:, h, i0:CHUNK],
                            start=(jb == 0), stop=(jb == njb - 1))
                # normalize: ot = po[0:64] * (1/sums) per head
                ot = sb.tile([128, CHUNK], F16, tag="ot", bufs=3,
                             name=f"ot{bidx}{ch}")
                rbs = []
                for h in range(HPC):
                    rrow = sb.tile([1, CHUNK], F32, tag="rrow", bufs=4,
                                   name=f"r{bidx}{ch}{h}")
                    nc.vector.reciprocal(rrow, pos[h][DH:DH + 1, :])
                    rb = sb.tile([DH, CHUNK], F32, tag="rb", bufs=4,
                                 name=f"rb{bidx}{ch}{h}")
                    nc.gpsimd.partition_broadcast(rb, rrow)
                    rbs.append(rb)
                for h in range(HPC):
                    nc.vector.tensor_mul(ot[h * 64:(h + 1) * 64, :],
                                         pos[h][0:DH, :], rbs[h])
                return ot

            def p2_proj(bidx, ch, ot, act_evac=False):
                """Deferred Wo projection + evacuation for one token chunk."""
                gcol = bidx * N + ch * CHUNK
                osb = sb.tile([128, NCB, CHUNK], F16, tag="osb", bufs=3,
                              name=f"o{bidx}{ch}")
                grp = 2 if act_evac else NCB // 2
                for db in range(NCB):
                    ppr = ps.tile([128, CHUNK], F32, tag="sm", bufs=2,
                                  name=f"pj{bidx}{ch}{db}")
                    nc.tensor.matmul(ppr, wo[:, db, :], ot,
                                     start=True, stop=True)
                    if act_evac and db % 2 == 0:
                        nc.scalar.copy(osb[:, db, :], ppr)
                    else:
                        nc.vector.tensor_copy(osb[:, db, :], ppr)
                    if db % grp == grp - 1:
                        hb = db // grp
                        nc.sync.dma_start(
                            out=outD[hb * grp * 128:(hb + 1) * grp * 128,
                                     gcol:gcol + CHUNK].rearrange(
                                "(db p) n -> p db n", p=128),
                            in_=osb[:, hb * grp:(hb + 1) * grp, :])

            for rep in range(reps):
                if rep > 0:
                    issue_xt(0)
                    issue_xt(4)
                    issue_xt(1)
                # interleave both batches' attention with QKV and deferred
                # projections so PE always has queued work while ACT runs exp
                p1_chunk(0)
                a00 = p2_attn(0, 0)
                p1_chunk(4)
                p1_chunk(1)
                a10 = p2_attn(1, 0)
                p2_proj(0, 0, a00)
                p1_chunk(5)
                a01 = p2_attn(0, 1)
                p2_proj(1, 0, a10)
                p1_chunk(2)
                a11 = p2_attn(1, 1)
                p2_proj(0, 1, a01)
                p1_chunk(6)
                a02 = p2_attn(0, 2)
                p2_proj(1, 1, a11)
                p1_chunk(3)
                a12 = p2_attn(1, 2)
                p2_proj(0, 2, a02)
                p1_chunk(7)
                a03 = p2_attn(0, 3)
                p2_proj(1, 2, a12)
                a13 = p2_attn(1, 3)
                p2_proj(0, 3, a03, act_evac=True)
                p2_proj(1, 3, a13, act_evac=True)
    nc.compile()
    return nc


def _get_nc(reps=1):
    if reps not in _NC_CACHE:
        _NC_CACHE[reps] = build(reps)
    return _NC_CACHE[reps]


def make_in_maps(x, pos_emb, Wq, Wk, Wv, Wo):
    x = np.asarray(x, np.float32)
    pos_emb = np.asarray(pos_emb, np.float32)
    scale = np.float32(DH ** -0.5)

    xT = np.ascontiguousarray(x.reshape(T, DIM).T).astype(np.float16)
    cosT = np.cos(pos_emb).T                       # [DH, N]
    sinT = np.sin(pos_emb).T
    sinsT = np.concatenate([-sinT[0:32], sinT[32:64]], axis=0)
    cos128 = np.tile(cosT, (2, 1)).astype(np.float16)      # [128, N]
    sins128 = np.tile(sinsT, (2, 1)).astype(np.float16)

    ident = np.eye(128, dtype=np.float16)
    jj = np.arange(128)[:, None]
    ii = np.arange(128)[None, :]
    mask = np.where(jj > ii, NEG, 0.0).astype(np.float16)

    in_maps = []
    for c in range(NCORES):
        cols = slice(c * 128, (c + 1) * 128)
        in_maps.append(dict(
            xT=xT,
            wq=(np.ascontiguousarray(Wq[:, cols]) * scale).astype(np.float16),
            wk=np.ascontiguousarray(Wk[:, cols]).astype(np.float16),
            wv=np.ascontiguousarray(Wv[:, cols]).astype(np.float16),
            wo=np.ascontiguousarray(Wo[cols, :]).astype(np.float16),
            cosT=cos128, sinsT=sins128, identD=ident, maskD=mask,
            identFD=np.eye(128, dtype=np.float32),
        ))
    return in_maps


def run(in_maps, trace=False, reps=1, **kw):
    nc = _get_nc(reps)
    return run_bass_kernel_spmd(nc, in_maps, list(range(NCORES)),
                                trace=trace, **kw)


def kernel(x, pos_emb, Wq, Wk, Wv, Wo, bo):
    in_maps = make_in_maps(x, pos_emb, Wq, Wk, Wv, Wo)
    res = run(in_maps)
    acc = np.zeros((DIM, T), np.float32)
    for c in range(NCORES):
        acc += res.results[c]["outT"].astype(np.float32)
    out = acc.T.reshape(B, N, DIM) + np.asarray(bo, np.float32)[None, None, :]
    return out.astype(np.float32)



# revision 37
# speedup vs baseline: 2.5196x; 2.5196x over previous
"""Causal multi-head attention with RoPE on 8 Trainium2 NeuronCores.

Problem: B=2, N=2048, DIM=1024, H=16, DH=64, fp32 in/out.
Sharding: head-parallel - core c owns heads 2c, 2c+1 (columns c*128:(c+1)*128
of Wq/Wk/Wv, rows c*128:(c+1)*128 of Wo) for both batches. Each core computes
its partial output projection [DIM, B*N] in fp16; the host sums the 8
partials (the "all-reduce") and adds the bias.

v2 vs baseline:
  - fp16 end-to-end on SBUF/HBM (fp32 only in PSUM accumulation): halves DMA
    and enables 2x DVE modes on SBUF-side elementwise ops.
  - V transposed per 128-token block as one [128,128] PE transpose covering
    both heads; v_aug interleaves [h0 64 | ones | h1 64 | ones] so one
    strided DVE copy scatters both heads.
  - causal diagonal handled at 128-col granularity: S/exp/PV skip the
    fully-masked region, mask add is a N=128 matmul with a shared [128,128]
    triangle pattern.
  - S matmuls for the two heads (K=64, partition bases 0/64) are emitted
    adjacently so they pack into disjoint PE row groups and run concurrently.
  - batch-0/batch-1 attention chunks interleave with QKV chunks and deferred
    Wo projections so PE always has queued work while ACT runs exp; each
    attention block is preceded in the PE queue by an independent QKV chunk
    so S matmuls never head-of-line block on the RoPE chain.
  - q/k PSUM evacuation on ScalarE; rotate-half via partition-shifted DVE
    copies; x chunks prefetched 3 slots ahead; one output store DMA per
    4 Wo blocks.

v3 vs v2:
  - softmax reciprocal via the custom-DVE reciprocal_approx_fast (~5x
    faster than the 8-cycle-per-element InstReciprocal, which cost 3.3us
    per [1,512] row and stalled the PE ~5.5us per attention block through
    the po PSUM-buffer recycle). The sums row is staged PSUM->SBUF on
    ScalarE first: the custom op reads garbage from PSUM directly (HW
    NaN; fine in CoreSim).
  - attn(1,3) hoisted before proj(1,2) so the projection matmuls fill the
    PE while the final block's normalize chain drains; p1(4) hoisted before
    attn(0,0) so the first S matmuls have a full QKV chunk of queue cover
    over chunk 0's RoPE latency.
  - ident loads first (scalar ring) and feeds 25 warm-up matmuls so the
    PE's HAM clock-gate reaches 8/8 during the DMA prologue; x chunks land
    as two half-cb tiles so QKV starts on the first 256KB.
  - next rep's first x chunks prefetched before the tail, and the next
    rep's chunk-0 QKV+RoPE woven into the current rep's ACT-bound tail
    (legal after attn(0,3): its qkt/v_aug writes only conflict with batch
    0) - the marginal rep measures ~154us traced vs ~193us standalone.

HW-validated rel err vs fp32 reference: 4.7e-04 (budget 2e-2).
Traced exec: ~193us reps=1 / ~154us marginal rep (baseline traced
242.4us). Note: the device thermally downclocks (P0, PE 2.4->2.0GHz)
after sustained back-to-back benching; cool-state numbers are ~5-35us
faster than hot-state ones.
"""
import numpy as np
import bass_rust
from concourse import bacc
import concourse.mybir as mybir
from concourse.tile import TileContext
from concourse.bass_utils import run_bass_kernel_spmd

B, N, DIM, H, DH = 2, 2048, 1024, 16, 64
NCORES = 8
HPC = H // NCORES          # 2 heads per core
T = B * N                  # 4096 tokens
CHUNK = 512
NCH = T // CHUNK           # 8 token chunks
NCB = DIM // 128           # 8 contraction blocks
NJB = N // 128             # 16 j-blocks per batch
NEG = -60000.0             # causal mask add (fp16-safe; exp -> 0 in fp32)

F32 = mybir.dt.float32
F16 = mybir.dt.float16

_NC_CACHE = {}


def build(reps=1):
    nc = bacc.Bacc()
    xTD = nc.dram_tensor("xT", [DIM, T], F16, kind="ExternalInput")
    wqD = nc.dram_tensor("wq", [DIM, 128], F16, kind="ExternalInput")
    wkD = nc.dram_tensor("wk", [DIM, 128], F16, kind="ExternalInput")
    wvD = nc.dram_tensor("wv", [DIM, 128], F16, kind="ExternalInput")
    woD = nc.dram_tensor("wo", [128, DIM], F16, kind="ExternalInput")
    cosD = nc.dram_tensor("cosT", [128, N], F16, kind="ExternalInput")
    sinsD = nc.dram_tensor("sinsT", [128, N], F16, kind="ExternalInput")
    identD = nc.dram_tensor("identD", [128, 128], F16, kind="ExternalInput")
    identFD = nc.dram_tensor("identFD", [128, 128], F32, kind="ExternalInput")
    maskD = nc.dram_tensor("maskD", [128, 128], F16, kind="ExternalInput")
    outD = nc.dram_tensor("outT", [DIM, T], F16, kind="ExternalOutput")

    Exp = mybir.ActivationFunctionType.Exp

    with TileContext(nc) as tc:
        with (
            tc.tile_pool(name="const", bufs=1) as cp,
            tc.tile_pool(name="sb", bufs=2) as sb,
            tc.tile_pool(name="ps", bufs=1, space="PSUM") as ps,
        ):
            ident = cp.tile([128, 128], F16)
            identF = cp.tile([128, 128], F32)
            mask128 = cp.tile([128, 128], F16)
            wq = cp.tile([128, NCB, 128], F16)
            wk = cp.tile([128, NCB, 128], F16)
            wv = cp.tile([128, NCB, 128], F16)
            wo = cp.tile([128, NCB, 128], F16)
            cos2 = cp.tile([128, N], F16)
            sins2 = cp.tile([128, N], F16)
            qkt = cp.tile([128, 2, T], F16)   # slot 0 = q, slot 1 = k
            v_aug = cp.tile([128, B, NJB, 130], F16)

            xt_tiles = {}

            def issue_xt(ch):
                """Prefetch one 512-token x chunk as two half-cb tiles so
                QKV matmuls on the first half start while the second
                streams."""
                t0 = ch * CHUNK
                half = NCB // 2
                parts = []
                for hb in range(2):
                    xt = sb.tile([128, half, CHUNK], F16, tag=f"xt{hb}",
                                 bufs=4, name=f"xt_c{ch}_{hb}")
                    nc.sync.dma_start(
                        out=xt,
                        in_=xTD[hb * half * 128:(hb + 1) * half * 128,
                                t0:t0 + CHUNK].rearrange(
                            "(cb p) n -> p cb n", p=128))
                    parts.append(xt)
                xt_tiles[ch] = parts

            # prologue: ident loads first (tiny, scalar ring) to feed
            # warm-up matmuls that hold the PE's HAM clock-gate at 8/8
            # while the real x/weight DMAs stream on the sync ring
            nc.scalar.dma_start(out=ident, in_=identD[:])
            warm = ps.tile([128, CHUNK], F32, tag="sm", bufs=2, name="warm")
            for _ in range(25):
                nc.tensor.matmul(warm[:, 0:128], ident, ident,
                                 start=True, stop=True)
            for hb in range(2):
                nc.sync.dma_start(
                    out=wq[:, hb * 4:(hb + 1) * 4, :],
                    in_=wqD[hb * 512:(hb + 1) * 512, :].rearrange(
                        "(cb p) d -> p cb d", p=128))
            nc.scalar.dma_start(out=mask128, in_=maskD[:])
            nc.vector.memset(
                v_aug.rearrange("p b j (two c) -> p b j two c", two=2)
                [:, :, :, :, 64:65], 1.0)
            issue_xt(0)
            nc.sync.dma_start(out=cos2[:, 0:CHUNK], in_=cosD[:, 0:CHUNK])
            nc.sync.dma_start(out=sins2[:, 0:CHUNK], in_=sinsD[:, 0:CHUNK])
            nc.sync.dma_start(
                out=wk, in_=wkD[:].rearrange("(cb p) d -> p cb d", p=128))
            issue_xt(4)
            nc.sync.dma_start(
                out=wv, in_=wvD[:].rearrange("(cb p) d -> p cb d", p=128))
            issue_xt(1)
            nc.sync.dma_start(out=cos2[:, CHUNK:], in_=cosD[:, CHUNK:])
            nc.sync.dma_start(out=sins2[:, CHUNK:], in_=sinsD[:, CHUNK:])
            nc.sync.dma_start(out=identF, in_=identFD[:])
            nc.gpsimd.dma_start(
                out=wo, in_=woD[:].rearrange("p (db d) -> p db d", d=128))

            # x chunks are consumed in order 0,4,1,5,2,6,3,7; prefetch the
            # chunk-after-next at the start of each p1 chunk
            _xt_order = [0, 4, 1, 5, 2, 6, 3, 7]

            def p1_chunk(ch):
                """QKV + RoPE + V transpose for one 512-token chunk."""
                t0 = ch * CHUNK
                bidx = t0 // N
                csl = slice(t0 % N, t0 % N + CHUNK)   # cos/sin columns
                nxt = _xt_order.index(ch) + 3
                if nxt < NCH:
                    issue_xt(_xt_order[nxt])
                xtp = xt_tiles.pop(ch)
                half = NCB // 2

                def xtb(cb):
                    return xtp[cb // half][:, cb % half, :]

                # q and k projections share one 2-bank PSUM tile
                pp = ps.tile([128, 2, CHUNK], F32, tag="big", bufs=2,
                             name=f"ppqk{ch}")
                for sl, W in ((0, wq), (1, wk)):
                    for cb in range(NCB):
                        nc.tensor.matmul(pp[:, sl, :], W[:, cb, :],
                                         xtb(cb),
                                         start=(cb == 0), stop=(cb == NCB - 1))
                raw = sb.tile([128, 2, CHUNK], F16, tag="raw", bufs=4,
                              name=f"raw{ch}")
                nc.scalar.copy(raw, pp)
                # rotate-half swap (sign folded into sins): partition-shifted
                # DVE copies covering both q and k slots
                rawsw = sb.tile([128, 2, CHUNK], F16, tag="rawsw", bufs=4,
                                name=f"rawsw{ch}")
                for hh in (0, 64):
                    a, bnd, c2 = hh, hh + 32, hh + 64
                    nc.vector.tensor_copy(rawsw[a:bnd], raw[bnd:c2])
                    nc.vector.tensor_copy(rawsw[bnd:c2], raw[a:bnd])
                # v projection (overlaps the q/k RoPE elementwise chain)
                ppv = ps.tile([128, 2, CHUNK], F32, tag="big", bufs=2,
                              name=f"ppv{ch}")
                for cb in range(NCB):
                    nc.tensor.matmul(ppv[:, 0, :], wv[:, cb, :], xtb(cb),
                                     start=(cb == 0), stop=(cb == NCB - 1))
                # evacuate v early so the ppv PSUM buffer frees before
                # the RoPE chain drains through the DVE queue
                vtc = sb.tile([128, CHUNK], F32, tag="vtc", bufs=2,
                              name=f"vtc{ch}")
                nc.vector.tensor_copy(vtc, ppv[:, 0, :])
                # RoPE: qkt = raw*cos + rawsw*sins
                for sl in range(2):
                    tmp = sb.tile([128, CHUNK], F16, tag="tmp", bufs=2,
                                  name=f"tmp{ch}{sl}")
                    tmp2 = sb.tile([128, CHUNK], F16, tag="tmp2", bufs=2,
                                   name=f"tmp2{ch}{sl}")
                    nc.vector.tensor_mul(tmp, raw[:, sl, :], cos2[:, csl])
                    nc.vector.tensor_mul(tmp2, rawsw[:, sl, :], sins2[:, csl])
                    nc.vector.tensor_add(qkt[:, sl, t0:t0 + CHUNK], tmp, tmp2)
                for tb in range(4):
                    jb = (ch % 4) * 4 + tb
                    pt = ps.tile([128, CHUNK], F32, tag="sm", bufs=2,
                                 name=f"pt{ch}{tb}")
                    nc.tensor.transpose(
                        pt[:, 0:128], vtc[:, tb * 128:(tb + 1) * 128],
                        identF)
                    nc.vector.tensor_copy(
                        v_aug[:, bidx, jb, :].rearrange(
                            "p (two c) -> p two c", two=2)[:, :, 0:64],
                        pt[:, 0:128].rearrange("p (two c) -> p two c", two=2))

            def p2_attn(bidx, ch):
                """S/exp/PV + softmax normalize for i-chunk ch of batch bidx.

                Returns the normalized attention output tile [128, CHUNK]
                (rows 0:64 head0, 64:128 head1) for the deferred projection.
                """
                gcol = bidx * N + ch * CHUNK
                njb = 4 * (ch + 1)
                pos = []
                for h in range(HPC):
                    pos.append(ps.tile([DH + 1, CHUNK], F32, tag="po", bufs=2,
                                       name=f"po{bidx}{ch}{h}"))
                expts = []
                for jb in range(njb):
                    r = jb - 4 * ch          # >=0 on the diagonal band
                    i0 = 128 * r if r > 0 else 0
                    jc = bidx * N + jb * 128
                    pst = ps.tile([128, 2, CHUNK], F32, tag="big", bufs=2,
                                  name=f"ps{bidx}{ch}{jb}")
                    diag = r >= 0
                    # head0 (rows 0:64) and head1 (rows 64:128) S matmuls
                    # pack into disjoint PE row groups
                    for h in range(HPC):
                        qr = slice(h * 64, (h + 1) * 64)
                        nc.tensor.matmul(
                            pst[:, h, i0:CHUNK], qkt[qr, 1, jc:jc + 128],
                            qkt[qr, 0, gcol + i0:gcol + CHUNK],
                            start=True, stop=not diag)
                    if diag:
                        for h in range(HPC):
                            nc.tensor.matmul(
                                pst[:, h, i0:i0 + 128], ident, mask128,
                                start=False, stop=True)
                    expt = sb.tile([128, 2, CHUNK], F16, tag="expt", bufs=8,
                                   name=f"e{bidx}{ch}{jb}")
                    nc.scalar.activation(expt[:, :, i0:CHUNK],
                                         pst[:, :, i0:CHUNK], Exp)
                    expts.append((jb, i0, expt))
                    for h in range(HPC):
                        nc.tensor.matmul(
                            pos[h][:, i0:CHUNK],
                            v_aug[:, bidx, jb, 65 * h:65 * h + 65],
                            expt[:, h, i0:CHUNK],
                            start=(jb == 0), stop=(jb == njb - 1))
                # normalize: ot = po[0:64] * (1/sums) per head
                ot = sb.tile([128, CHUNK], F16, tag="ot", bufs=3,
                             name=f"ot{bidx}{ch}")
                rbs = []
                for h in range(HPC):
                    srow = sb.tile([1, CHUNK], F32, tag="srow", bufs=4,
                                   name=f"s{bidx}{ch}{h}")
                    nc.scalar.copy(srow, pos[h][DH:DH + 1, :])
                    rrow = sb.tile([1, CHUNK], F32, tag="rrow", bufs=4,
                                   name=f"r{bidx}{ch}{h}")
                    nc.vector.reciprocal_approx_fast(rrow, srow)
                    rb = sb.tile([DH, CHUNK], F32, tag="rb", bufs=4,
                                 name=f"rb{bidx}{ch}{h}")
                    nc.gpsimd.partition_broadcast(rb, rrow)
                    rbs.append(rb)
                for h in range(HPC):
                    nc.vector.tensor_mul(ot[h * 64:(h + 1) * 64, :],
                                         pos[h][0:DH, :], rbs[h])
                return ot

            def p2_proj(bidx, ch, ot, act_evac=False):
                """Deferred Wo projection + evacuation for one token chunk."""
                gcol = bidx * N + ch * CHUNK
                osb = sb.tile([128, NCB, CHUNK], F16, tag="osb", bufs=3,
                              name=f"o{bidx}{ch}")
                grp = 2 if act_evac else NCB // 2
                for db in range(NCB):
                    ppr = ps.tile([128, CHUNK], F32, tag="sm", bufs=2,
                                  name=f"pj{bidx}{ch}{db}")
                    nc.tensor.matmul(ppr, wo[:, db, :], ot,
                                     start=True, stop=True)
                    if act_evac and db % 2 == 0:
                        nc.scalar.copy(osb[:, db, :], ppr)
                    else:
                        nc.vector.tensor_copy(osb[:, db, :], ppr)
                    if db % grp == grp - 1:
                        hb = db // grp
                        nc.sync.dma_start(
                            out=outD[hb * grp * 128:(hb + 1) * grp * 128,
                                     gcol:gcol + CHUNK].rearrange(
                                "(db p) n -> p db n", p=128),
                            in_=osb[:, hb * grp:(hb + 1) * grp, :])

            for rep in range(reps):
                # interleave both batches' attention with QKV and deferred
                # projections so PE always has queued work while ACT runs exp
                if rep == 0:
                    p1_chunk(0)
                # (for rep>0, chunk 0 was woven into the previous rep's tail)
                p1_chunk(4)
                a00 = p2_attn(0, 0)
                p1_chunk(1)
                a10 = p2_attn(1, 0)
                p2_proj(0, 0, a00)
                p1_chunk(5)
                a01 = p2_attn(0, 1)
                p2_proj(1, 0, a10)
                p1_chunk(2)
                a11 = p2_attn(1, 1)
                p2_proj(0, 1, a01)
                p1_chunk(6)
                a02 = p2_attn(0, 2)
                p2_proj(1, 1, a11)
                p1_chunk(3)
                a12 = p2_attn(1, 2)
                p2_proj(0, 2, a02)
                p1_chunk(7)
                if rep + 1 < reps:
                    # prefetch next rep's first x chunks before the tail so
                    # the next rep's QKV starts without a DMA wait
                    issue_xt(0)
                    issue_xt(4)
                    issue_xt(1)
                a03 = p2_attn(0, 3)
                if rep + 1 < reps:
                    # weave the next rep's first QKV chunk into this rep's
                    # ACT-bound tail: its qkt/v_aug writes only conflict
                    # with a03 (batch 0), which is complete by here
                    p1_chunk(0)
                a13 = p2_attn(1, 3)
                p2_proj(1, 2, a12)
                p2_proj(0, 3, a03, act_evac=True)
                p2_proj(1, 3, a13, act_evac=True)
    nc.compile()
    return nc


def _get_nc(reps=1):
    if reps not in _NC_CACHE:
        _NC_CACHE[reps] = build(reps)
    return _NC_CACHE[reps]


def make_in_maps(x, pos_emb, Wq, Wk, Wv, Wo):
    x = np.asarray(x, np.float32)
    pos_emb = np.asarray(pos_emb, np.float32)
    scale = np.float32(DH ** -0.5)

    xT = np.ascontiguousarray(x.reshape(T, DIM).T).astype(np.float16)
    cosT = np.cos(pos_emb).T                       # [DH, N]
    sinT = np.sin(pos_emb).T
    sinsT = np.concatenate([-sinT[0:32], sinT[32:64]], axis=0)
    cos128 = np.tile(cosT, (2, 1)).astype(np.float16)      # [128, N]
    sins128 = np.tile(sinsT, (2, 1)).astype(np.float16)

    ident = np.eye(128, dtype=np.float16)
    jj = np.arange(128)[:, None]
    ii = np.arange(128)[None, :]
    mask = np.where(jj > ii, NEG, 0.0).astype(np.float16)

    in_maps = []
    for c in range(NCORES):
        cols = slice(c * 128, (c + 1) * 128)
        in_maps.append(dict(
            xT=xT,
            wq=(np.ascontiguousarray(Wq[:, cols]) * scale).astype(np.float16),
            wk=np.ascontiguousarray(Wk[:, cols]).astype(np.float16),
            wv=np.ascontiguousarray(Wv[:, cols]).astype(np.float16),
            wo=np.ascontiguousarray(Wo[cols, :]).astype(np.float16),
            cosT=cos128, sinsT=sins128, identD=ident, maskD=mask,
            identFD=np.eye(128, dtype=np.float32),
        ))
    return in_maps


def run(in_maps, trace=False, reps=1, **kw):
    nc = _get_nc(reps)
    return run_bass_kernel_spmd(nc, in_maps, list(range(NCORES)),
                                trace=trace, **kw)


def kernel(x, pos_emb, Wq, Wk, Wv, Wo, bo):
    in_maps = make_in_maps(x, pos_emb, Wq, Wk, Wv, Wo)
    res = run(in_maps)
    acc = np.zeros((DIM, T), np.float32)
    for c in range(NCORES):
        acc += res.results[c]["outT"].astype(np.float32)
    out = acc.T.reshape(B, N, DIM) + np.asarray(bo, np.float32)[None, None, :]
    return out.astype(np.float32)

